# revision 19
# baseline (speedup 1.0000x reference)
"""Trainium2 Bass kernel for EnhancedMultiHeadAttention.

Full (unsharded) inputs in, full output out. Internally: Megatron-style
tensor parallelism over heads — 16 heads across 8 cores = 2 heads/core.

Per-core device program (matmul inputs bf16, fp32 PSUM/softmax), one
continuous software-pipelined stream:
  - pre: load everything; project q/k for batch 0 (transposed space)
  - attention runs as 8 chunks of (batch, 512 queries); each chunk is
    [QK x16 j-tiles, 2 heads row-packed at PE tiles (0,0)/(64,0) -> exp ->
    *exp(mask) band] then [PV x16 with ones row] with the REMAINING
    projection / transpose / output-projection work injected into the PV
    phases (where the PE has slack), so there are no idle phase holes.
  - softmax normalization by 1/Z via DRAM-bounce partition broadcast.
  - output projection consumes attention output directly (it is already
    the stationary-operand layout) -> partial^T in DRAM; host sums 8
    partials and adds bo.
"""

import numpy as np
import ml_dtypes

import concourse.bass as bass
import concourse.mybir as mybir
import concourse.tile as tile
from concourse import bacc
from concourse.bass_utils import run_bass_kernel_spmd
from concourse.masks import make_identity

B, S, D, H, DK = 2, 2048, 1024, 16, 64
NCORES = 8
HPC = H // NCORES            # heads per core = 2
EC = HPC * DK                # features per core = 128
T = B * S                    # tokens = 4096
KT = D // 128                # contraction k-tiles = 8
NS = S // 512                # 512-token slices per batch = 4
NJ = S // 128                # j-tiles per batch = 16
SUB = 8                      # j-tiles per QK/PV sub-phase
WINDOW_SIZES = (5, 10, 20, 40)
BAND = 243                   # exp(-(d^2)/(2*40^2)) < 1e-8 beyond this
BF16 = mybir.dt.bfloat16
F32 = mybir.dt.float32
NPBF16 = ml_dtypes.bfloat16
OUT_BF16 = True

_EXP = mybir.ActivationFunctionType.Exp


def _band_tiles():
    out = []
    for jt in range(NJ):
        for ic in range(NS):
            j0, i0 = jt * 128, ic * 512
            if i0 < j0 + 128 + BAND and i0 + 512 > j0:
                out.append((jt, ic))
    return out


BAND_TILES = _band_tiles()
BAND_IDX = {ji: n for n, ji in enumerate(BAND_TILES)}
NBAND = len(BAND_TILES)


def _exp_mask_T():
    """exp(maskT) band tiles, packed [128, NBAND, 512] bf16 (partition-major)."""
    i = np.arange(S, dtype=np.float32)[:, None]
    j = np.arange(S, dtype=np.float32)[None, :]
    d2 = (i - j) ** 2
    lower = j <= i
    m = sum(
        np.where(lower, np.exp(-d2 / np.float32(2.0 * w * w)), np.float32(0.0))
        for w in WINDOW_SIZES
    ) / np.float32(len(WINDOW_SIZES))
    em = np.exp(m.T)  # exp(maskT[j, i])
    packed = np.empty((128, NBAND, 512), dtype=NPBF16)
    for (jt, ic), n in BAND_IDX.items():
        packed[:, n, :] = em[jt * 128:(jt + 1) * 128,
                             ic * 512:(ic + 1) * 512].astype(NPBF16)
    return packed


def build_program(reps=1, timing=False, timing_out=False):
    nc = bacc.Bacc("TRN2", target_bir_lowering=False, debug=False, num_devices=NCORES)

    out_dt = BF16 if OUT_BF16 else F32
    kin = "Internal" if timing else "ExternalInput"
    kout = "Internal" if (timing and not timing_out) else "ExternalOutput"
    xq = nc.dram_tensor("xq", [128, KT, T], BF16, kind=kin)
    xk = nc.dram_tensor("xk", [128, KT, T], BF16, kind=kin)
    xv = nc.dram_tensor("xv", [128, KT, T], BF16, kind=kin)
    wq = nc.dram_tensor("wq", [128, KT, EC], BF16, kind=kin)
    wk = nc.dram_tensor("wk", [128, KT, EC], BF16, kind=kin)
    wv = nc.dram_tensor("wv", [128, KT, EC], BF16, kind=kin)
    wo = nc.dram_tensor("wo", [EC, D], BF16, kind=kin)
    bq = nc.dram_tensor("bq", [EC, 1], F32, kind=kin)
    bk = nc.dram_tensor("bk", [EC, 1], F32, kind=kin)
    bv = nc.dram_tensor("bv", [EC, 1], F32, kind=kin)
    em = nc.dram_tensor("em", [128, NBAND, 512], BF16, kind=kin)
    out_pt = nc.dram_tensor("out_pt", [D, T], out_dt, kind=kout)
    if timing:
        tiny = nc.dram_tensor("tiny", [1, 8], F32, kind="ExternalOutput")
    rz_dram = nc.dram_tensor("rz_scratch", [B * NS * HPC, 512], F32)

    with tile.TileContext(nc) as tc:
      for _rep in range(reps):
        with (
            tc.tile_pool(name="persist", bufs=1) as persist,
            tc.tile_pool(name="xt", bufs=18) as xt_pool,
            tc.tile_pool(name="work", bufs=2) as work,
            tc.tile_pool(name="drain", bufs=6) as drain_pool,
            tc.tile_pool(name="pp", bufs=2, space="PSUM") as pp,
            tc.tile_pool(name="sc", bufs=2, space="PSUM") as sc_pool,
            tc.tile_pool(name="pv", bufs=2, space="PSUM") as pv_pool,
        ):
            # ---- persistent SBUF ----
            qTt = {(bb, n): persist.tile([128, 512], BF16, tag=f"qT{bb}_{n}",
                                         name=f"qT{bb}_{n}")
                   for bb in range(B) for n in range(NS)}
            kTt = {(bb, n): persist.tile([128, 512], BF16, tag=f"kT{bb}_{n}",
                                         name=f"kT{bb}_{n}")
                   for bb in range(B) for n in range(NS)}
            vTt = {(bb, n): persist.tile([128, 512], BF16, tag=f"vT{bb}_{n}",
                                         name=f"vT{bb}_{n}")
                   for bb in range(B) for n in range(NS)}
            v_all = [persist.tile([128, HPC * (DK + 1)], BF16, tag=f"v{tt}",
                                  name=f"v{tt}") for tt in range(T // 128)]
            attn = [persist.tile([128, 512], BF16, tag=f"attn{gg}",
                                 name=f"attn{gg}") for gg in range(T // 512)]
            wq_sb = persist.tile([128, KT, EC], BF16, tag="wq")
            wk_sb = persist.tile([128, KT, EC], BF16, tag="wk")
            wv_sb = persist.tile([128, KT, EC], BF16, tag="wv")
            wo_sb = persist.tile([EC, D], BF16, tag="wo")
            bq_sb = persist.tile([EC, 1], F32, tag="bq")
            bk_sb = persist.tile([EC, 1], F32, tag="bk")
            bv_sb = persist.tile([EC, 1], F32, tag="bv")
            em_sb = persist.tile([128, NBAND, 512], BF16, tag="em")
            ident = persist.tile([128, 128], BF16, tag="ident")

            # ---- constant + bulk loads (two HWDGE queues, b0 first) ----
            nc.scalar.dma_start(out=wq_sb, in_=wq[:])
            nc.scalar.dma_start(out=wk_sb, in_=wk[:])
            nc.scalar.dma_start(out=wv_sb, in_=wv[:])
            nc.scalar.dma_start(out=wo_sb, in_=wo[:])
            nc.scalar.dma_start(out=bq_sb, in_=bq[:])
            nc.scalar.dma_start(out=bk_sb, in_=bk[:])
            nc.scalar.dma_start(out=bv_sb, in_=bv[:])
            make_identity(nc, ident)
            for tt in range(T // 128):
                ones_ap = v_all[tt].rearrange("p (h x) -> p h x", h=HPC)
                nc.vector.memset(ones_ap[:, :, DK:DK + 1], 1.0)

            x_sb = {}

            def load_x(bb, xi, eng):
                xd = (xq, xk, xv)[xi]
                for k in range(KT):
                    t = xt_pool.tile([128, S], BF16, tag="xt",
                                     name=f"x{bb}_{xi}_{k}")
                    eng.dma_start(out=t, in_=xd[:, k, bb * S:(bb + 1) * S])
                    x_sb[(bb, xi, k)] = t

            load_x(0, 0, nc.sync)     # xq b0
            load_x(0, 1, nc.scalar)   # xk b0
            load_x(0, 2, nc.sync)     # xv b0
            nc.sync.dma_start(out=em_sb, in_=em[:])
            load_x(1, 0, nc.scalar)   # xq b1
            load_x(1, 1, nc.sync)     # xk b1
            load_x(1, 2, nc.scalar)   # xv b1

            # ---- emit helpers ----
            wsbs = (wq_sb, wk_sb, wv_sb)
            bsbs = (bq_sb, bk_sb, bv_sb)
            outs = (qTt, kTt, vTt)

            def emit_proj_slice(bb, xi, n):
                ps = pp.tile([128, 512], F32, tag="ps", name="ps")
                for k in range(KT):
                    nc.tensor.matmul(
                        ps,
                        wsbs[xi][:, k, :],
                        x_sb[(bb, xi, k)][:, n * 512:(n + 1) * 512],
                        start=(k == 0),
                        stop=(k == KT - 1),
                    )
                nc.vector.tensor_scalar_add(
                    out=outs[xi][(bb, n)], in0=ps, scalar1=bsbs[xi]
                )

            def emit_transpose(bb, tl):
                tt = bb * NJ + tl
                pst = pp.tile([128, 128], BF16, tag="ps", name="pst")
                n, o = divmod(tl * 128, 512)
                nc.tensor.transpose(pst, vTt[(bb, n)][:, o:o + 128], ident)
                dst = v_all[tt].rearrange("p (h x) -> p h x", h=HPC)[:, :, 0:DK]
                nc.vector.tensor_copy(
                    out=dst, in_=pst.rearrange("p (h x) -> p h x", h=HPC)
                )

            ndr = [0]

            def emit_outproj(fm, gg, roff=0):
                po = pp.tile([128, 512], F32, tag="ps", name="po")
                nc.tensor.matmul(
                    po,
                    wo_sb[:, fm * 128:(fm + 1) * 128],
                    attn[gg][:],
                    start=True,
                    stop=True,
                )
                og = drain_pool.tile([128, 512], out_dt, tag="og", name="og")
                t0 = (gg * 512 + roff) % T
                nc.vector.tensor_copy(out=og, in_=po)
                eng = nc.sync if ndr[0] % 2 == 0 else nc.scalar
                eng.dma_start(
                    out=out_pt[fm * 128:(fm + 1) * 128, t0:t0 + 512], in_=og)
                ndr[0] += 1

            # ---- pre-phase: q/k projections for batch 0 ----
            emit_proj_slice(0, 0, 0)
            for n in range(NS):
                emit_proj_slice(0, 1, n)
            for n in range(1, NS):
                emit_proj_slice(0, 0, n)

            # ---- injection schedule: chunk (b, it) -> {jt: [thunk, ...]} ----
            roff = (_rep * 512) % T if timing else 0
            sched = {}

            def add(b, it, jt, fn):
                sched.setdefault((b, it), {}).setdefault(jt, []).append(fn)

            for jt in range(NJ):
                if jt % 4 == 0:
                    add(0, 0, jt, (lambda n: lambda: emit_proj_slice(0, 2, n))(jt // 4))
                add(0, 0, jt, (lambda t: lambda: emit_transpose(0, t))(jt))
            for jt in range(NJ):
                if jt % 2 == 0:
                    i = jt // 2
                    xi, n = (0, i // 2) if i % 2 == 0 else (1, i // 2)
                    add(0, 1, jt, (lambda a, b_: lambda: emit_proj_slice(1, a, b_))(xi, n))
            for jt in range(NJ):
                if jt % 4 == 0:
                    add(0, 2, jt, (lambda n: lambda: emit_proj_slice(1, 2, n))(jt // 4))
                add(0, 3, jt, (lambda t: lambda: emit_transpose(1, t))(jt))
            op_items = [(fm, gg) for fm in range(D // 128) for gg in range(NS)]
            pos = 0
            for it in (1, 2, 3):
                for jt in range(NJ):
                    if pos < len(op_items):
                        fm, gg = op_items[pos]
                        add(1, it, jt, (lambda f, g: lambda: emit_outproj(f, g, roff))(fm, gg))
                        pos += 1

            # ---- attention: 8 chunks, clustered QK then PV(+injections) ----
            for b in range(B):
                for it in range(NS):
                    gg = b * NS + it
                    inj = sched.get((b, it), {})
                    pv_ps = [pv_pool.tile([DK + 1, 512], F32, tag="pv",
                                          name=f"pv{h}") for h in range(HPC)]
                    ets = {}
                    for j0 in range(0, NJ, SUB):
                        for jt in range(j0, j0 + SUB):
                            # both heads' scoresT in one 2-bank tile -> 1 exp
                            sp = sc_pool.tile([128, 2 * 512], F32, tag="sc",
                                              name="sc")
                            for h in range(HPC):
                                hp = slice(64 * h, 64 * h + 64)
                                nc.tensor.matmul(
                                    sp[:, h * 512:(h + 1) * 512],
                                    kTt[(b, jt // 4)][hp, (jt % 4) * 128:
                                                      (jt % 4) * 128 + 128],
                                    qTt[(b, it)][hp, :],
                                    start=True,
                                    stop=True,
                                )
                            et = work.tile([128, 2 * 512], BF16, tag="et",
                                           name="et", bufs=SUB + 2)
                            nc.scalar.activation(
                                out=et, in_=sp, func=_EXP,
                                scale=1.0 / np.sqrt(DK)
                            )
                            if (jt, it) in BAND_IDX:
                                bi = BAND_IDX[(jt, it)]
                                for h in range(HPC):
                                    nc.vector.tensor_mul(
                                        out=et[:, h * 512:(h + 1) * 512],
                                        in0=et[:, h * 512:(h + 1) * 512],
                                        in1=em_sb[:, bi, :],
                                    )
                            ets[jt] = et
                        for jt in range(j0, j0 + SUB):
                            for fn in inj.get(jt, ()):
                                fn()
                            for h in range(HPC):
                                nc.tensor.matmul(
                                    pv_ps[h],
                                    v_all[b * NJ + jt][
                                        :, h * (DK + 1):(h + 1) * (DK + 1)],
                                    ets[jt][:, h * 512:(h + 1) * 512],
                                    start=(jt == 0),
                                    stop=(jt == NJ - 1),
                                )
                    # normalize: out^T[e, i] * (1/Z[i])
                    for h in range(HPC):
                        ridx = gg * HPC + h
                        rz = work.tile([DK + 1, 512], F32, tag="rz", name="rz",
                                       bufs=2)
                        nc.vector.reciprocal(
                            out=rz[DK:DK + 1, :], in_=pv_ps[h][DK:DK + 1, :]
                        )
                        nc.sync.dma_start(out=rz_dram[ridx, :],
                                          in_=rz[DK:DK + 1, :])
                        rzb = work.tile([64, 512], F32, tag="rzb", name="rzb",
                                        bufs=2)
                        nc.sync.dma_start(
                            out=rzb,
                            in_=rz_dram[ridx:ridx + 1, :].to_broadcast([64, 512]),
                        )
                        if h == 0:
                            nc.vector.tensor_mul(
                                out=attn[gg][0:64, :], in0=pv_ps[h][0:DK, :],
                                in1=rzb
                            )
                        else:
                            stg = work.tile([64, 512], BF16, tag="stg",
                                            name="stg", bufs=2)
                            nc.vector.tensor_mul(
                                out=stg, in0=pv_ps[h][0:DK, :], in1=rzb
                            )
                            nc.sync.dma_start(out=attn[gg][64:128, :], in_=stg)

            # ---- tail: remaining output projection (batch 1) ----
            for fm in range(D // 128):
                for gg in range(NS, 2 * NS):
                    emit_outproj(fm, gg, roff)

            if timing and _rep == reps - 1:
                tt_ = work.tile([1, 8], F32, tag="tiny", name="tiny")
                nc.vector.tensor_copy(out=tt_, in_=qTt[(0, 0)][0:1, 0:8])
                nc.sync.dma_start(out=tiny[:], in_=tt_)

    nc.compile()
    return nc


def _pack_xt(x):
    # [B, S, D] f32 -> [128, KT, T] bf16 partition-major X^T
    xt = x.reshape(T, KT, 128).transpose(2, 1, 0)
    return np.ascontiguousarray(xt.astype(NPBF16))


def _prep_inputs(Q, K, V, Wq, bq, Wk, bk, Wv, bv, Wo, bo):
    """Build per-core input maps (host-side shard + transpose + cast)."""
    em_packed = _exp_mask_T()
    xq, xk, xv = _pack_xt(Q), _pack_xt(K), _pack_xt(V)
    in_maps = []
    for c in range(NCORES):
        sl = slice(EC * c, EC * (c + 1))

        def wpack(W):
            wt = W[sl, :].T.reshape(KT, 128, EC).transpose(1, 0, 2)
            return np.ascontiguousarray(wt.astype(NPBF16))

        in_maps.append({
            "xq": xq, "xk": xk, "xv": xv,
            "wq": wpack(Wq), "wk": wpack(Wk), "wv": wpack(Wv),
            "wo": np.ascontiguousarray(Wo[:, sl].T.astype(NPBF16)),
            "bq": bq[sl].reshape(EC, 1).astype(np.float32),
            "bk": bk[sl].reshape(EC, 1).astype(np.float32),
            "bv": bv[sl].reshape(EC, 1).astype(np.float32),
            "em": em_packed,
        })
    return in_maps


_NC_CACHE = []


def _get_nc():
    if not _NC_CACHE:
        _NC_CACHE.append(build_program())
    return _NC_CACHE[0]


def kernel(Q, K, V, Wq, bq, Wk, bk, Wv, bv, Wo, bo):
    nc = _get_nc()
    in_maps = _prep_inputs(Q, K, V, Wq, bq, Wk, bk, Wv, bv, Wo, bo)
    res = run_bass_kernel_spmd(nc, in_maps, core_ids=list(range(NCORES)))
    total = np.zeros((D, T), np.float32)
    for c in range(NCORES):
        total += res.results[c]["out_pt"].astype(np.float32)
    out = total.T + bo.astype(np.float32)
    return np.ascontiguousarray(out.reshape(B, S, D))


# revision 20
# speedup vs baseline: 92.8698x; 92.8698x over previous
"""Trainium2 Bass kernel for EnhancedMultiHeadAttention.

Full (unsharded) inputs in, full output out. Internally: Megatron-style
tensor parallelism over heads — 16 heads across 8 cores = 2 heads/core.

Per-core device program (matmul inputs bf16, fp32 PSUM/softmax), one
continuous software-pipelined stream:
  - pre: load everything; project q/k for batch 0 (transposed space)
  - attention runs as 8 chunks of (batch, 512 queries); each chunk is
    [QK x16 j-tiles, 2 heads row-packed at PE tiles (0,0)/(64,0) -> exp ->
    *exp(mask) band] then [PV x16 with ones row] with the REMAINING
    projection / transpose / output-projection work injected into the PV
    phases (where the PE has slack), so there are no idle phase holes.
  - softmax normalization by 1/Z via DRAM-bounce partition broadcast.
  - output projection consumes attention output directly (it is already
    the stationary-operand layout) -> partial^T in DRAM; host sums 8
    partials and adds bo.
"""

import numpy as np
import ml_dtypes

import concourse.bass as bass
import concourse.mybir as mybir
import concourse.tile as tile
from concourse import bacc
from concourse.bass_utils import run_bass_kernel_spmd
from concourse.masks import make_identity

B, S, D, H, DK = 2, 2048, 1024, 16, 64
NCORES = 8
HPC = H // NCORES            # heads per core = 2
EC = HPC * DK                # features per core = 128
T = B * S                    # tokens = 4096
KT = D // 128                # contraction k-tiles = 8
NS = S // 512                # 512-token slices per batch = 4
NJ = S // 128                # j-tiles per batch = 16
SUB = 8                      # j-tiles per QK/PV sub-phase
WINDOW_SIZES = (5, 10, 20, 40)
BAND = 243                   # exp(-(d^2)/(2*40^2)) < 1e-8 beyond this
BF16 = mybir.dt.bfloat16
F32 = mybir.dt.float32
NPBF16 = ml_dtypes.bfloat16
OUT_BF16 = True

_EXP = mybir.ActivationFunctionType.Exp


def _band_tiles():
    out = []
    for jt in range(NJ):
        for ic in range(NS):
            j0, i0 = jt * 128, ic * 512
            if i0 < j0 + 128 + BAND and i0 + 512 > j0:
                out.append((jt, ic))
    return out


BAND_TILES = _band_tiles()
BAND_IDX = {ji: n for n, ji in enumerate(BAND_TILES)}
NBAND = len(BAND_TILES)


def _exp_mask_T():
    """exp(maskT) band tiles, packed [128, NBAND, 512] bf16 (partition-major)."""
    i = np.arange(S, dtype=np.float32)[:, None]
    j = np.arange(S, dtype=np.float32)[None, :]
    d2 = (i - j) ** 2
    lower = j <= i
    m = sum(
        np.where(lower, np.exp(-d2 / np.float32(2.0 * w * w)), np.float32(0.0))
        for w in WINDOW_SIZES
    ) / np.float32(len(WINDOW_SIZES))
    em = np.exp(m.T)  # exp(maskT[j, i])
    packed = np.empty((128, NBAND, 512), dtype=NPBF16)
    for (jt, ic), n in BAND_IDX.items():
        packed[:, n, :] = em[jt * 128:(jt + 1) * 128,
                             ic * 512:(ic + 1) * 512].astype(NPBF16)
    return packed


def build_program(reps=1, timing=False, timing_out=False):
    nc = bacc.Bacc("TRN2", target_bir_lowering=False, debug=False, num_devices=NCORES)

    out_dt = BF16 if OUT_BF16 else F32
    kin = "Internal" if timing else "ExternalInput"
    kout = "Internal" if (timing and not timing_out) else "ExternalOutput"
    xq = nc.dram_tensor("xq", [128, KT, T], BF16, kind=kin)
    xk = nc.dram_tensor("xk", [128, KT, T], BF16, kind=kin)
    xv = nc.dram_tensor("xv", [128, KT, T], BF16, kind=kin)
    wq = nc.dram_tensor("wq", [128, KT, EC], BF16, kind=kin)
    wk = nc.dram_tensor("wk", [128, KT, EC], BF16, kind=kin)
    wv = nc.dram_tensor("wv", [128, KT, EC], BF16, kind=kin)
    wo = nc.dram_tensor("wo", [EC, D], BF16, kind=kin)
    bq = nc.dram_tensor("bq", [EC, 1], F32, kind=kin)
    bk = nc.dram_tensor("bk", [EC, 1], F32, kind=kin)
    bv = nc.dram_tensor("bv", [EC, 1], F32, kind=kin)
    em = nc.dram_tensor("em", [128, NBAND, 512], BF16, kind=kin)
    out_pt = nc.dram_tensor("out_pt", [D, T], out_dt, kind=kout)
    if timing:
        tiny = nc.dram_tensor("tiny", [1, 8], F32, kind="ExternalOutput")
    rz_dram = nc.dram_tensor("rz_scratch", [B * NS * HPC, 512], F32)

    with tile.TileContext(nc) as tc:
      for _rep in range(reps):
        with (
            tc.tile_pool(name="persist", bufs=1) as persist,
            tc.tile_pool(name="xt", bufs=18) as xt_pool,
            tc.tile_pool(name="work", bufs=2) as work,
            tc.tile_pool(name="drain", bufs=6) as drain_pool,
            tc.tile_pool(name="pp", bufs=2, space="PSUM") as pp,
            tc.tile_pool(name="sc", bufs=2, space="PSUM") as sc_pool,
            tc.tile_pool(name="pv", bufs=2, space="PSUM") as pv_pool,
        ):
            # ---- persistent SBUF ----
            qTt = {(bb, n): persist.tile([128, 512], BF16, tag=f"qT{bb}_{n}",
                                         name=f"qT{bb}_{n}")
                   for bb in range(B) for n in range(NS)}
            kTt = {(bb, n): persist.tile([128, 512], BF16, tag=f"kT{bb}_{n}",
                                         name=f"kT{bb}_{n}")
                   for bb in range(B) for n in range(NS)}
            vTt = {(bb, n): persist.tile([128, 512], BF16, tag=f"vT{bb}_{n}",
                                         name=f"vT{bb}_{n}")
                   for bb in range(B) for n in range(NS)}
            v_all = [persist.tile([128, HPC * (DK + 1)], BF16, tag=f"v{tt}",
                                  name=f"v{tt}") for tt in range(T // 128)]
            attn = [persist.tile([128, 512], BF16, tag=f"attn{gg}",
                                 name=f"attn{gg}") for gg in range(T // 512)]
            wq_sb = persist.tile([128, KT, EC], BF16, tag="wq")
            wk_sb = persist.tile([128, KT, EC], BF16, tag="wk")
            wv_sb = persist.tile([128, KT, EC], BF16, tag="wv")
            wo_sb = persist.tile([EC, D], BF16, tag="wo")
            bq_sb = persist.tile([EC, 1], F32, tag="bq")
            bk_sb = persist.tile([EC, 1], F32, tag="bk")
            bv_sb = persist.tile([EC, 1], F32, tag="bv")
            em_sb = persist.tile([128, NBAND, 512], BF16, tag="em")
            ident = persist.tile([128, 128], BF16, tag="ident")

            # ---- constant + bulk loads (two HWDGE queues, b0 first) ----
            nc.scalar.dma_start(out=wq_sb, in_=wq[:])
            nc.scalar.dma_start(out=wk_sb, in_=wk[:])
            nc.scalar.dma_start(out=wv_sb, in_=wv[:])
            nc.scalar.dma_start(out=wo_sb, in_=wo[:])
            nc.scalar.dma_start(out=bq_sb, in_=bq[:])
            nc.scalar.dma_start(out=bk_sb, in_=bk[:])
            nc.scalar.dma_start(out=bv_sb, in_=bv[:])
            make_identity(nc, ident)
            for tt in range(T // 128):
                ones_ap = v_all[tt].rearrange("p (h x) -> p h x", h=HPC)
                nc.vector.memset(ones_ap[:, :, DK:DK + 1], 1.0)

            x_sb = {}

            def load_x(bb, xi, eng):
                xd = (xq, xk, xv)[xi]
                for k in range(KT):
                    t = xt_pool.tile([128, S], BF16, tag="xt",
                                     name=f"x{bb}_{xi}_{k}")
                    eng.dma_start(out=t, in_=xd[:, k, bb * S:(bb + 1) * S])
                    x_sb[(bb, xi, k)] = t

            load_x(0, 0, nc.sync)     # xq b0
            load_x(0, 1, nc.scalar)   # xk b0
            load_x(0, 2, nc.sync)     # xv b0
            nc.sync.dma_start(out=em_sb, in_=em[:])
            load_x(1, 0, nc.scalar)   # xq b1
            load_x(1, 1, nc.sync)     # xk b1
            load_x(1, 2, nc.scalar)   # xv b1

            # ---- emit helpers ----
            wsbs = (wq_sb, wk_sb, wv_sb)
            bsbs = (bq_sb, bk_sb, bv_sb)
            outs = (qTt, kTt, vTt)

            def emit_proj_slice(bb, xi, n):
                ps = pp.tile([128, 512], F32, tag="ps", name="ps")
                for k in range(KT):
                    nc.tensor.matmul(
                        ps,
                        wsbs[xi][:, k, :],
                        x_sb[(bb, xi, k)][:, n * 512:(n + 1) * 512],
                        start=(k == 0),
                        stop=(k == KT - 1),
                    )
                nc.vector.tensor_scalar_add(
                    out=outs[xi][(bb, n)], in0=ps, scalar1=bsbs[xi]
                )

            def emit_transpose(bb, tl):
                tt = bb * NJ + tl
                pst = pp.tile([128, 128], BF16, tag="ps", name="pst")
                n, o = divmod(tl * 128, 512)
                nc.tensor.transpose(pst, vTt[(bb, n)][:, o:o + 128], ident)
                dst = v_all[tt].rearrange("p (h x) -> p h x", h=HPC)[:, :, 0:DK]
                nc.vector.tensor_copy(
                    out=dst, in_=pst.rearrange("p (h x) -> p h x", h=HPC)
                )

            ndr = [0]

            def emit_outproj(fm, gg, roff=0):
                po = pp.tile([128, 512], F32, tag="ps", name="po")
                nc.tensor.matmul(
                    po,
                    wo_sb[:, fm * 128:(fm + 1) * 128],
                    attn[gg][:],
                    start=True,
                    stop=True,
                )
                og = drain_pool.tile([128, 512], out_dt, tag="og", name="og")
                t0 = (gg * 512 + roff) % T
                nc.vector.tensor_copy(out=og, in_=po)
                eng = nc.sync if ndr[0] % 2 == 0 else nc.scalar
                eng.dma_start(
                    out=out_pt[fm * 128:(fm + 1) * 128, t0:t0 + 512], in_=og)
                ndr[0] += 1

            # ---- pre-phase: batch-0 q slice 0 + all k slices ----
            emit_proj_slice(0, 0, 0)
            for n in range(NS):
                emit_proj_slice(0, 1, n)

            # ---- injection schedule: chunk (b, it) -> {jt: [thunk, ...]} ----
            roff = (_rep * 512) % T if timing else 0
            sched = {}

            def add(b, it, jt, fn):
                sched.setdefault((b, it), {}).setdefault(jt, []).append(fn)

            for jt in range(NJ):
                if jt % 4 == 0:
                    add(0, 0, jt, (lambda n: lambda: emit_proj_slice(0, 2, n))(jt // 4))
                if jt % 4 == 2 and jt // 4 < 3:
                    add(0, 0, jt, (lambda n: lambda: emit_proj_slice(0, 0, n))(jt // 4 + 1))
                add(0, 0, jt, (lambda t: lambda: emit_transpose(0, t))(jt))
            for jt in range(NJ):
                if jt % 2 == 0:
                    i = jt // 2
                    xi, n = (0, i // 2) if i % 2 == 0 else (1, i // 2)
                    add(0, 1, jt, (lambda a, b_: lambda: emit_proj_slice(1, a, b_))(xi, n))
            for jt in range(NJ):
                if jt % 4 == 0:
                    add(0, 2, jt, (lambda n: lambda: emit_proj_slice(1, 2, n))(jt // 4))
                add(0, 3, jt, (lambda t: lambda: emit_transpose(1, t))(jt))
            op_sched = {0: (0, 1), 1: (2, 3), 2: (4, 5), 3: (6,)}
            for it, ggs in op_sched.items():
                for i, gg in enumerate(ggs):
                    for fm in range(D // 128):
                        add(1, it, 2 * fm + i,
                            (lambda f, g: lambda: emit_outproj(f, g, roff))(fm, gg))

            # ---- attention: 8 chunks, clustered QK then PV(+injections) ----
            for b in range(B):
                for it in range(NS):
                    gg = b * NS + it
                    inj = sched.get((b, it), {})
                    pv_ps = [pv_pool.tile([DK + 1, 512], F32, tag="pv",
                                          name=f"pv{h}") for h in range(HPC)]
                    ets = {}
                    for j0 in range(0, NJ, SUB):
                        for jt in range(j0, j0 + SUB):
                            # both heads' scoresT in one 2-bank tile -> 1 exp
                            sp = sc_pool.tile([128, 2 * 512], F32, tag="sc",
                                              name="sc")
                            for h in range(HPC):
                                hp = slice(64 * h, 64 * h + 64)
                                nc.tensor.matmul(
                                    sp[:, h * 512:(h + 1) * 512],
                                    kTt[(b, jt // 4)][hp, (jt % 4) * 128:
                                                      (jt % 4) * 128 + 128],
                                    qTt[(b, it)][hp, :],
                                    start=True,
                                    stop=True,
                                )
                            et = work.tile([128, 2 * 512], BF16, tag="et",
                                           name="et", bufs=SUB + 2)
                            nc.scalar.activation(
                                out=et, in_=sp, func=_EXP,
                                scale=1.0 / np.sqrt(DK)
                            )
                            if (jt, it) in BAND_IDX:
                                bi = BAND_IDX[(jt, it)]
                                for h in range(HPC):
                                    nc.vector.tensor_mul(
                                        out=et[:, h * 512:(h + 1) * 512],
                                        in0=et[:, h * 512:(h + 1) * 512],
                                        in1=em_sb[:, bi, :],
                                    )
                            ets[jt] = et
                        for jt in range(j0, j0 + SUB):
                            for fn in inj.get(jt, ()):
                                fn()
                            for h in range(HPC):
                                nc.tensor.matmul(
                                    pv_ps[h],
                                    v_all[b * NJ + jt][
                                        :, h * (DK + 1):(h + 1) * (DK + 1)],
                                    ets[jt][:, h * 512:(h + 1) * 512],
                                    start=(jt == 0),
                                    stop=(jt == NJ - 1),
                                )
                    # normalize: out^T[e, i] * (1/Z[i])
                    for h in range(HPC):
                        ridx = gg * HPC + h
                        rz = work.tile([DK + 1, 512], F32, tag="rz", name="rz",
                                       bufs=2)
                        nc.vector.reciprocal(
                            out=rz[DK:DK + 1, :], in_=pv_ps[h][DK:DK + 1, :]
                        )
                        ub = work.tile([DK, 512], BF16, tag="ub", name="ub",
                                       bufs=4)
                        nc.vector.tensor_copy(out=ub, in_=pv_ps[h][0:DK, :])
                        nc.sync.dma_start(out=rz_dram[ridx, :],
                                          in_=rz[DK:DK + 1, :])
                        rzb = work.tile([64, 512], F32, tag="rzb", name="rzb",
                                        bufs=2)
                        nc.sync.dma_start(
                            out=rzb,
                            in_=rz_dram[ridx:ridx + 1, :].to_broadcast([64, 512]),
                        )
                        if h == 0:
                            nc.vector.tensor_mul(
                                out=attn[gg][0:64, :], in0=ub, in1=rzb
                            )
                        else:
                            stg = work.tile([64, 512], BF16, tag="stg",
                                            name="stg", bufs=2)
                            nc.vector.tensor_mul(out=stg, in0=ub, in1=rzb)
                            nc.sync.dma_start(out=attn[gg][64:128, :], in_=stg)

            # ---- tail: last output projection slice ----
            for fm in range(D // 128):
                emit_outproj(fm, 2 * NS - 1, roff)

            if timing and _rep == reps - 1:
                tt_ = work.tile([1, 8], F32, tag="tiny", name="tiny")
                nc.vector.tensor_copy(out=tt_, in_=qTt[(0, 0)][0:1, 0:8])
                nc.sync.dma_start(out=tiny[:], in_=tt_)

    nc.compile()
    return nc


def _pack_xt(x):
    # [B, S, D] f32 -> [128, KT, T] bf16 partition-major X^T
    xt = x.reshape(T, KT, 128).transpose(2, 1, 0)
    return np.ascontiguousarray(xt.astype(NPBF16))


def _prep_inputs(Q, K, V, Wq, bq, Wk, bk, Wv, bv, Wo, bo):
    """Build per-core input maps (host-side shard + transpose + cast)."""
    em_packed = _exp_mask_T()
    xq, xk, xv = _pack_xt(Q), _pack_xt(K), _pack_xt(V)
    in_maps = []
    for c in range(NCORES):
        sl = slice(EC * c, EC * (c + 1))

        def wpack(W):
            wt = W[sl, :].T.reshape(KT, 128, EC).transpose(1, 0, 2)
            return np.ascontiguousarray(wt.astype(NPBF16))

        in_maps.append({
            "xq": xq, "xk": xk, "xv": xv,
            "wq": wpack(Wq), "wk": wpack(Wk), "wv": wpack(Wv),
            "wo": np.ascontiguousarray(Wo[:, sl].T.astype(NPBF16)),
            "bq": bq[sl].reshape(EC, 1).astype(np.float32),
            "bk": bk[sl].reshape(EC, 1).astype(np.float32),
            "bv": bv[sl].reshape(EC, 1).astype(np.float32),
            "em": em_packed,
        })
    return in_maps


_NC_CACHE = []


def _get_nc():
    if not _NC_CACHE:
        _NC_CACHE.append(build_program())
    return _NC_CACHE[0]


def kernel(Q, K, V, Wq, bq, Wk, bk, Wv, bv, Wo, bo):
    nc = _get_nc()
    in_maps = _prep_inputs(Q, K, V, Wq, bq, Wk, bk, Wv, bv, Wo, bo)
    res = run_bass_kernel_spmd(nc, in_maps, core_ids=list(range(NCORES)))
    total = np.zeros((D, T), np.float32)
    for c in range(NCORES):
        total += res.results[c]["out_pt"].astype(np.float32)
    out = total.T + bo.astype(np.float32)
    return np.ascontiguousarray(out.reshape(B, S, D))


# revision 22
# speedup vs baseline: 96.1247x; 1.0350x over previous
"""Trainium2 Bass kernel for EnhancedMultiHeadAttention.

Full (unsharded) inputs in, full output out. Internally: Megatron-style
tensor parallelism over heads — 16 heads across 8 cores = 2 heads/core.

Per-core device program (matmul inputs bf16, fp32 PSUM/softmax), one
continuous software-pipelined stream:
  - pre: load everything; project q/k for batch 0 (transposed space)
  - attention runs as 8 chunks of (batch, 512 queries); each chunk is
    [QK x16 j-tiles, 2 heads row-packed at PE tiles (0,0)/(64,0) -> exp ->
    *exp(mask) band] then [PV x16 with ones row] with the REMAINING
    projection / transpose / output-projection work injected into the PV
    phases (where the PE has slack), so there are no idle phase holes.
  - softmax normalization by 1/Z via DRAM-bounce partition broadcast.
  - output projection consumes attention output directly (it is already
    the stationary-operand layout) -> partial^T in DRAM; host sums 8
    partials and adds bo.
"""

import numpy as np
import ml_dtypes

import concourse.bass as bass
import concourse.mybir as mybir
import concourse.tile as tile
from concourse import bacc
from concourse.bass_utils import run_bass_kernel_spmd
from concourse.masks import make_identity

B, S, D, H, DK = 2, 2048, 1024, 16, 64
NCORES = 8
HPC = H // NCORES            # heads per core = 2
EC = HPC * DK                # features per core = 128
T = B * S                    # tokens = 4096
KT = D // 128                # contraction k-tiles = 8
NS = S // 512                # 512-token slices per batch = 4
NJ = S // 128                # j-tiles per batch = 16
SUB = 8                      # j-tiles per QK/PV sub-phase
WINDOW_SIZES = (5, 10, 20, 40)
BAND = 243                   # exp(-(d^2)/(2*40^2)) < 1e-8 beyond this
BF16 = mybir.dt.bfloat16
F32 = mybir.dt.float32
NPBF16 = ml_dtypes.bfloat16
OUT_BF16 = True

_EXP = mybir.ActivationFunctionType.Exp


def _band_tiles():
    out = []
    for jt in range(NJ):
        for ic in range(NS):
            j0, i0 = jt * 128, ic * 512
            if i0 < j0 + 128 + BAND and i0 + 512 > j0:
                out.append((jt, ic))
    return out


BAND_TILES = _band_tiles()
BAND_IDX = {ji: n for n, ji in enumerate(BAND_TILES)}
NBAND = len(BAND_TILES)


def _exp_mask_T():
    """exp(maskT) band tiles, packed [128, NBAND, 512] bf16 (partition-major)."""
    i = np.arange(S, dtype=np.float32)[:, None]
    j = np.arange(S, dtype=np.float32)[None, :]
    d2 = (i - j) ** 2
    lower = j <= i
    m = sum(
        np.where(lower, np.exp(-d2 / np.float32(2.0 * w * w)), np.float32(0.0))
        for w in WINDOW_SIZES
    ) / np.float32(len(WINDOW_SIZES))
    em = np.exp(m.T)  # exp(maskT[j, i])
    packed = np.empty((128, NBAND, 512), dtype=NPBF16)
    for (jt, ic), n in BAND_IDX.items():
        packed[:, n, :] = em[jt * 128:(jt + 1) * 128,
                             ic * 512:(ic + 1) * 512].astype(NPBF16)
    return packed


def build_program(reps=1, timing=False, timing_out=False):
    nc = bacc.Bacc("TRN2", target_bir_lowering=False, debug=False, num_devices=NCORES)

    out_dt = BF16 if OUT_BF16 else F32
    kin = "Internal" if timing else "ExternalInput"
    kout = "Internal" if (timing and not timing_out) else "ExternalOutput"
    xq = nc.dram_tensor("xq", [128, KT, T], BF16, kind=kin)
    xk = nc.dram_tensor("xk", [128, KT, T], BF16, kind=kin)
    xv = nc.dram_tensor("xv", [128, KT, T], BF16, kind=kin)
    wq = nc.dram_tensor("wq", [128, KT, EC], BF16, kind=kin)
    wk = nc.dram_tensor("wk", [128, KT, EC], BF16, kind=kin)
    wv = nc.dram_tensor("wv", [128, KT, EC], BF16, kind=kin)
    wo = nc.dram_tensor("wo", [EC, D], BF16, kind=kin)
    bq = nc.dram_tensor("bq", [EC, 1], F32, kind=kin)
    bk = nc.dram_tensor("bk", [EC, 1], F32, kind=kin)
    bv = nc.dram_tensor("bv", [EC, 1], F32, kind=kin)
    em = nc.dram_tensor("em", [128, NBAND, 512], BF16, kind=kin)
    out_pt = nc.dram_tensor("out_pt", [D, T], out_dt, kind=kout)
    if timing:
        tiny = nc.dram_tensor("tiny", [1, 8], F32, kind="ExternalOutput")
    rz_dram = nc.dram_tensor("rz_scratch", [B * NS * HPC, 512], F32)

    with tile.TileContext(nc) as tc:
      for _rep in range(reps):
        with (
            tc.tile_pool(name="persist", bufs=1) as persist,
            tc.tile_pool(name="xt", bufs=18) as xt_pool,
            tc.tile_pool(name="work", bufs=2) as work,
            tc.tile_pool(name="drain", bufs=6) as drain_pool,
            tc.tile_pool(name="pp", bufs=2, space="PSUM") as pp,
            tc.tile_pool(name="sc", bufs=2, space="PSUM") as sc_pool,
            tc.tile_pool(name="pv", bufs=2, space="PSUM") as pv_pool,
        ):
            # ---- persistent SBUF ----
            qTt = {(bb, n): persist.tile([128, 512], BF16, tag=f"qT{bb}_{n}",
                                         name=f"qT{bb}_{n}")
                   for bb in range(B) for n in range(NS)}
            kTt = {(bb, n): persist.tile([128, 512], BF16, tag=f"kT{bb}_{n}",
                                         name=f"kT{bb}_{n}")
                   for bb in range(B) for n in range(NS)}
            vTt = {(bb, n): persist.tile([128, 512], BF16, tag=f"vT{bb}_{n}",
                                         name=f"vT{bb}_{n}")
                   for bb in range(B) for n in range(NS)}
            v_all = [persist.tile([128, HPC * (DK + 1)], BF16, tag=f"v{tt}",
                                  name=f"v{tt}") for tt in range(T // 128)]
            attn = [persist.tile([128, 512], BF16, tag=f"attn{gg}",
                                 name=f"attn{gg}") for gg in range(T // 512)]
            wq_sb = persist.tile([128, KT, EC], BF16, tag="wq")
            wk_sb = persist.tile([128, KT, EC], BF16, tag="wk")
            wv_sb = persist.tile([128, KT, EC], BF16, tag="wv")
            wo_sb = persist.tile([EC, D], BF16, tag="wo")
            bq_sb = persist.tile([EC, 1], F32, tag="bq")
            bk_sb = persist.tile([EC, 1], F32, tag="bk")
            bv_sb = persist.tile([EC, 1], F32, tag="bv")
            em_sb = persist.tile([128, NBAND, 512], BF16, tag="em")
            ident = persist.tile([128, 128], BF16, tag="ident")

            # ---- constant + bulk loads (two HWDGE queues, b0 first) ----
            nc.scalar.dma_start(out=wq_sb, in_=wq[:])
            nc.scalar.dma_start(out=wk_sb, in_=wk[:])
            nc.scalar.dma_start(out=wv_sb, in_=wv[:])
            nc.scalar.dma_start(out=wo_sb, in_=wo[:])
            nc.scalar.dma_start(out=bq_sb, in_=bq[:])
            nc.scalar.dma_start(out=bk_sb, in_=bk[:])
            nc.scalar.dma_start(out=bv_sb, in_=bv[:])
            make_identity(nc, ident)
            for tt in range(T // 128):
                ones_ap = v_all[tt].rearrange("p (h x) -> p h x", h=HPC)
                nc.vector.memset(ones_ap[:, :, DK:DK + 1], 1.0)

            x_sb = {}

            def load_x(bb, xi, eng):
                xd = (xq, xk, xv)[xi]
                for k in range(KT):
                    t = xt_pool.tile([128, S], BF16, tag="xt",
                                     name=f"x{bb}_{xi}_{k}")
                    eng.dma_start(out=t, in_=xd[:, k, bb * S:(bb + 1) * S])
                    x_sb[(bb, xi, k)] = t

            load_x(0, 0, nc.sync)     # xq b0
            load_x(0, 1, nc.scalar)   # xk b0
            load_x(0, 2, nc.sync)     # xv b0
            nc.sync.dma_start(out=em_sb, in_=em[:])
            load_x(1, 0, nc.scalar)   # xq b1
            load_x(1, 1, nc.sync)     # xk b1
            load_x(1, 2, nc.scalar)   # xv b1

            # ---- emit helpers ----
            wsbs = (wq_sb, wk_sb, wv_sb)
            bsbs = (bq_sb, bk_sb, bv_sb)
            outs = (qTt, kTt, vTt)

            def emit_proj_slice(bb, xi, n):
                ps = pp.tile([128, 512], F32, tag="ps", name="ps")
                for k in range(KT):
                    nc.tensor.matmul(
                        ps,
                        wsbs[xi][:, k, :],
                        x_sb[(bb, xi, k)][:, n * 512:(n + 1) * 512],
                        start=(k == 0),
                        stop=(k == KT - 1),
                    )
                nc.vector.tensor_scalar_add(
                    out=outs[xi][(bb, n)], in0=ps, scalar1=bsbs[xi]
                )

            def emit_transpose(bb, tl):
                tt = bb * NJ + tl
                pst = pp.tile([128, 128], BF16, tag="ps", name="pst")
                n, o = divmod(tl * 128, 512)
                nc.tensor.transpose(pst, vTt[(bb, n)][:, o:o + 128], ident)
                dst = v_all[tt].rearrange("p (h x) -> p h x", h=HPC)[:, :, 0:DK]
                nc.vector.tensor_copy(
                    out=dst, in_=pst.rearrange("p (h x) -> p h x", h=HPC)
                )

            ndr = [0]

            def emit_outproj(fm, gg, roff=0):
                po = pp.tile([128, 512], F32, tag="ps", name="po")
                nc.tensor.matmul(
                    po,
                    wo_sb[:, fm * 128:(fm + 1) * 128],
                    attn[gg][:],
                    start=True,
                    stop=True,
                )
                og = drain_pool.tile([128, 512], out_dt, tag="og", name="og")
                t0 = (gg * 512 + roff) % T
                if ndr[0] % 2 == 0:
                    nc.vector.tensor_copy(out=og, in_=po)
                else:
                    nc.scalar.copy(out=og, in_=po)
                eng = nc.sync if ndr[0] % 2 == 0 else nc.scalar
                eng.dma_start(
                    out=out_pt[fm * 128:(fm + 1) * 128, t0:t0 + 512], in_=og)
                ndr[0] += 1

            # ---- pre-phase: batch-0 q slice 0 + all k slices ----
            emit_proj_slice(0, 0, 0)
            for n in range(NS):
                emit_proj_slice(0, 1, n)

            # ---- injection schedule: chunk (b, it) -> {jt: [thunk, ...]} ----
            roff = (_rep * 512) % T if timing else 0
            sched = {}

            def add(b, it, jt, fn):
                sched.setdefault((b, it), {}).setdefault(jt, []).append(fn)

            for jt in range(NJ):
                if jt % 4 == 0:
                    add(0, 0, jt, (lambda n: lambda: emit_proj_slice(0, 2, n))(jt // 4))
                if jt % 4 == 2 and jt // 4 < 3:
                    add(0, 0, jt, (lambda n: lambda: emit_proj_slice(0, 0, n))(jt // 4 + 1))
                add(0, 0, jt, (lambda t: lambda: emit_transpose(0, t))(jt))
            for jt in range(NJ):
                if jt % 2 == 0:
                    i = jt // 2
                    xi, n = (0, i // 2) if i % 2 == 0 else (1, i // 2)
                    add(0, 1, jt, (lambda a, b_: lambda: emit_proj_slice(1, a, b_))(xi, n))
            for jt in range(NJ):
                if jt % 4 == 0:
                    add(0, 2, jt, (lambda n: lambda: emit_proj_slice(1, 2, n))(jt // 4))
                add(0, 3, jt, (lambda t: lambda: emit_transpose(1, t))(jt))
            op_sched = {0: (0, 1), 1: (2, 3), 2: (4, 5), 3: (6,)}
            for it, ggs in op_sched.items():
                for i, gg in enumerate(ggs):
                    for fm in range(D // 128):
                        add(1, it, 2 * fm + i,
                            (lambda f, g: lambda: emit_outproj(f, g, roff))(fm, gg))

            # ---- attention: 8 chunks, clustered QK then PV(+injections) ----
            for b in range(B):
                for it in range(NS):
                    gg = b * NS + it
                    inj = sched.get((b, it), {})
                    pv_ps = [pv_pool.tile([DK + 1, 512], F32, tag="pv",
                                          name=f"pv{h}") for h in range(HPC)]
                    ets = {}
                    for j0 in range(0, NJ, SUB):
                        for jt in range(j0, j0 + SUB):
                            # both heads' scoresT in one 2-bank tile -> 1 exp
                            sp = sc_pool.tile([128, 2 * 512], F32, tag="sc",
                                              name="sc")
                            for h in range(HPC):
                                hp = slice(64 * h, 64 * h + 64)
                                nc.tensor.matmul(
                                    sp[:, h * 512:(h + 1) * 512],
                                    kTt[(b, jt // 4)][hp, (jt % 4) * 128:
                                                      (jt % 4) * 128 + 128],
                                    qTt[(b, it)][hp, :],
                                    start=True,
                                    stop=True,
                                )
                            et = work.tile([128, 2 * 512], BF16, tag="et",
                                           name="et", bufs=SUB + 2)
                            nc.scalar.activation(
                                out=et, in_=sp, func=_EXP,
                                scale=1.0 / np.sqrt(DK)
                            )
                            if (jt, it) in BAND_IDX:
                                bi = BAND_IDX[(jt, it)]
                                for h in range(HPC):
                                    nc.vector.tensor_mul(
                                        out=et[:, h * 512:(h + 1) * 512],
                                        in0=et[:, h * 512:(h + 1) * 512],
                                        in1=em_sb[:, bi, :],
                                    )
                            ets[jt] = et
                        for jt in range(j0, j0 + SUB):
                            for fn in inj.get(jt, ()):
                                fn()
                            for h in range(HPC):
                                nc.tensor.matmul(
                                    pv_ps[h],
                                    v_all[b * NJ + jt][
                                        :, h * (DK + 1):(h + 1) * (DK + 1)],
                                    ets[jt][:, h * 512:(h + 1) * 512],
                                    start=(jt == 0),
                                    stop=(jt == NJ - 1),
                                )
                    # normalize: out^T[e, i] * (1/Z[i])
                    for h in range(HPC):
                        ridx = gg * HPC + h
                        rz = work.tile([DK + 1, 512], F32, tag="rz", name="rz",
                                       bufs=2)
                        nc.vector.reciprocal(
                            out=rz[DK:DK + 1, :], in_=pv_ps[h][DK:DK + 1, :]
                        )
                        ub = work.tile([DK, 512], BF16, tag="ub", name="ub",
                                       bufs=4)
                        nc.vector.tensor_copy(out=ub, in_=pv_ps[h][0:DK, :])
                        nc.sync.dma_start(out=rz_dram[ridx, :],
                                          in_=rz[DK:DK + 1, :])
                        rzb = work.tile([64, 512], F32, tag="rzb", name="rzb",
                                        bufs=2)
                        nc.sync.dma_start(
                            out=rzb,
                            in_=rz_dram[ridx:ridx + 1, :].to_broadcast([64, 512]),
                        )
                        if h == 0:
                            nc.vector.tensor_mul(
                                out=attn[gg][0:64, :], in0=ub, in1=rzb
                            )
                        else:
                            stg = work.tile([64, 512], BF16, tag="stg",
                                            name="stg", bufs=2)
                            nc.vector.tensor_mul(out=stg, in0=ub, in1=rzb)
                            nc.sync.dma_start(out=attn[gg][64:128, :], in_=stg)

            # ---- tail: last output projection slice ----
            for fm in range(D // 128):
                emit_outproj(fm, 2 * NS - 1, roff)

            if timing and _rep == reps - 1:
                tt_ = work.tile([1, 8], F32, tag="tiny", name="tiny")
                nc.vector.tensor_copy(out=tt_, in_=qTt[(0, 0)][0:1, 0:8])
                nc.sync.dma_start(out=tiny[:], in_=tt_)

    nc.compile()
    return nc


def _pack_xt(x):
    # [B, S, D] f32 -> [128, KT, T] bf16 partition-major X^T
    xt = x.reshape(T, KT, 128).transpose(2, 1, 0)
    return np.ascontiguousarray(xt.astype(NPBF16))


def _prep_inputs(Q, K, V, Wq, bq, Wk, bk, Wv, bv, Wo, bo):
    """Build per-core input maps (host-side shard + transpose + cast)."""
    em_packed = _exp_mask_T()
    xq, xk, xv = _pack_xt(Q), _pack_xt(K), _pack_xt(V)
    in_maps = []
    for c in range(NCORES):
        sl = slice(EC * c, EC * (c + 1))

        def wpack(W):
            wt = W[sl, :].T.reshape(KT, 128, EC).transpose(1, 0, 2)
            return np.ascontiguousarray(wt.astype(NPBF16))

        in_maps.append({
            "xq": xq, "xk": xk, "xv": xv,
            "wq": wpack(Wq), "wk": wpack(Wk), "wv": wpack(Wv),
            "wo": np.ascontiguousarray(Wo[:, sl].T.astype(NPBF16)),
            "bq": bq[sl].reshape(EC, 1).astype(np.float32),
            "bk": bk[sl].reshape(EC, 1).astype(np.float32),
            "bv": bv[sl].reshape(EC, 1).astype(np.float32),
            "em": em_packed,
        })
    return in_maps


_NC_CACHE = []


def _get_nc():
    if not _NC_CACHE:
        _NC_CACHE.append(build_program())
    return _NC_CACHE[0]


def kernel(Q, K, V, Wq, bq, Wk, bk, Wv, bv, Wo, bo):
    nc = _get_nc()
    in_maps = _prep_inputs(Q, K, V, Wq, bq, Wk, bk, Wv, bv, Wo, bo)
    res = run_bass_kernel_spmd(nc, in_maps, core_ids=list(range(NCORES)))
    total = np.zeros((D, T), np.float32)
    for c in range(NCORES):
        total += res.results[c]["out_pt"].astype(np.float32)
    out = total.T + bo.astype(np.float32)
    return np.ascontiguousarray(out.reshape(B, S, D))
